# revision 4
# baseline (speedup 1.0000x reference)
# Trainium2 Bass kernel for nn_LiquidMalwareDetector.
#
# Strategy: pure data parallelism over the batch dim (8192 -> 1024 per core,
# 8 cores). Host side only reshapes/shards (layout, no arithmetic); all math
# runs on device:
#   phase 3: precompute clipped time c and c^2 for the gate (DRAM staging;
#            emitted first, scalar-queue DMAs so it overlaps phase 1)
#   phase 1: per-core BN channel sums/sumsq (matmul-with-0/1-matrix reduction)
#            + 8-core AllReduce of the 6 partial stats
#   phase 2: fold BN affine into the x-columns of the ff weights + bias
#   phase 4: 1024-step recurrence in feature-major layout, two independent
#            512-column batch chains interleaved to hide the serial latency.
#            State is the unsummed blend pair rz=[g*t1; (1-g)*t2]; duplicated
#            Wh rows in the K=128 matmul sum the pair (walrus forbids SBUF
#            tensor_tensor with mismatched base partitions, so the matmul
#            does the cross-partition add). Gate: K=2 matmul with +/-
#            stacked rows -> sigmoid gives [g; 1-g]. The two chains' gate
#            preacts share one 2-bank PSUM tile so a single [128,1024]
#            SIGMOID serves both (the scalar engine is the bottleneck:
#            3 ACT instructions/step instead of 4). The x-part matmul is
#            issued before the h-part so only the K=128 matmul sits on the
#            serial ACT->blend->matmul->ACT loop.
#   phase 5: classifier via sigmoid of logit-difference (== 2-class softmax)
import numpy as np

NUM_CORES = 8
B_FULL = 8192
S_FULL = 1024
F = 3
H = 64
BN_EPS = 1e-5

_CACHE = {}


def _build(num_cores, s_steps, b_loc):
    from concourse import bacc, mybir
    import concourse.tile as tile

    f16 = mybir.dt.float16
    f32 = mybir.dt.float32
    Alu = mybir.AluOpType
    Act = mybir.ActivationFunctionType

    NG = b_loc // 512  # 512-column groups per step
    assert b_loc % 512 == 0

    nc = bacc.Bacc(
        "TRN2",
        target_bir_lowering=False,
        debug=False,
        num_devices=num_cores,
    )

    # ---- I/O -------------------------------------------------------------
    xT = nc.dram_tensor("xT", [s_steps * F, b_loc], f16, kind="ExternalInput")
    tT = nc.dram_tensor("tT", [s_steps, b_loc], f16, kind="ExternalInput")
    # wzdup: [128,128] duplicated h-weights (rows 0-63 == rows 64-127 == Wh.T)
    # so the matmul itself sums the unsummed blend halves [u1; u2].
    wzdup_d = nc.dram_tensor("wzdup", [2 * H, 2 * H], f16, kind="ExternalInput")
    wzx_d = nc.dram_tensor("wzx", [F, 2 * H], f16, kind="ExternalInput")
    idup_d = nc.dram_tensor("idup", [2 * H, H], f16, kind="ExternalInput")
    bz_d = nc.dram_tensor("bz", [2 * H, 1], f32, kind="ExternalInput")
    wg2_d = nc.dram_tensor("wg2", [2, 2 * H], f16, kind="ExternalInput")
    cgb2_d = nc.dram_tensor("cgb2", [2 * H, 1], f32, kind="ExternalInput")
    eT_d = nc.dram_tensor("eT", [128, 9], f16, kind="ExternalInput")
    gam_d = nc.dram_tensor("gam", [F, 1], f32, kind="ExternalInput")
    bet_d = nc.dram_tensor("bet", [F, 1], f32, kind="ExternalInput")
    wcls_d = nc.dram_tensor("wcls", [H, 2], f16, kind="ExternalInput")
    bcls_d = nc.dram_tensor("bcls", [2, 1], f32, kind="ExternalInput")
    probs_d = nc.dram_tensor("probs", [2, b_loc], f32, kind="ExternalOutput")

    inv_bs = 1.0 / float(num_cores * b_loc * s_steps)

    with tile.TileContext(nc) as tc:
        with tc.tile_pool(name="const", bufs=1) as cpool, \
             tc.tile_pool(name="dram", bufs=1, space="DRAM") as dpool:
            # persistent SBUF-resident weights/constants
            wzdup = cpool.tile([2 * H, 2 * H], f16)
            nc.sync.dma_start(wzdup[:], wzdup_d[:])
            wzx = cpool.tile([F, 2 * H], f16)
            nc.sync.dma_start(wzx[:], wzx_d[:])
            # folded x-weights, placed at partitions 64..66 so the x matmul's
            # lhsT/rhs base partitions match (and auto row-group = 2)
            wzx6 = cpool.tile([H + F + 1, 2 * H], f16)
            idup = cpool.tile([2 * H, H], f16)
            nc.sync.dma_start(idup[:], idup_d[:])
            bz = cpool.tile([2 * H, 1], f32)
            nc.sync.dma_start(bz[:], bz_d[:])
            wg2 = cpool.tile([2, 2 * H], f16)
            nc.sync.dma_start(wg2[:], wg2_d[:])
            cgb2 = cpool.tile([2 * H, 1], f32)
            nc.sync.dma_start(cgb2[:], cgb2_d[:])
            eT = cpool.tile([128, 9], f16)
            nc.sync.dma_start(eT[:], eT_d[:])
            gam = cpool.tile([F, 1], f32)
            nc.sync.dma_start(gam[:], gam_d[:])
            bet = cpool.tile([F, 1], f32)
            nc.sync.dma_start(bet[:], bet_d[:])
            wcls = cpool.tile([H, 2], f16)
            nc.sync.dma_start(wcls[:], wcls_d[:])
            bcls = cpool.tile([2, 1], f32)
            nc.sync.dma_start(bcls[:], bcls_d[:])
            zb_f = cpool.tile([2 * H, 1], f32)   # folded tanh bias
            stats_g = cpool.tile([F, 2], f32)    # all-reduced [sum, sumsq]

            # ---- phase 3: gate input precompute (c, c^2) -----------------
            # Emitted first (no deps) with scalar-queue DMAs so its traffic
            # overlaps phase 1's sync-queue loads.
            cgd = dpool.tile([s_steps, 2 * b_loc], f16)
            with tc.tile_pool(name="cg", bufs=3) as cgp:
                n_ct = (s_steps + 127) // 128
                for i in range(n_ct):
                    r0 = i * 128
                    nr = min(128, s_steps - r0)
                    ttl = cgp.tile([128, b_loc], f16, tag="tt")
                    nc.scalar.dma_start(ttl[0:nr, :], tT[r0:r0 + nr, :])
                    cc2 = cgp.tile([128, 2 * b_loc], f16, tag="cc2")
                    nc.vector.tensor_scalar(cc2[0:nr, 0:b_loc], ttl[0:nr, :],
                                            60.0, 0.0, Alu.min, Alu.max)
                    nc.vector.tensor_tensor(
                        cc2[0:nr, b_loc:2 * b_loc], cc2[0:nr, 0:b_loc],
                        cc2[0:nr, 0:b_loc], Alu.mult)
                    nc.scalar.dma_start(cgd[r0:r0 + nr, :], cc2[0:nr, :])

            # ---- phase 1: BN stats ---------------------------------------
            rows_total = s_steps * F
            with tc.tile_pool(name="st_ps", bufs=1, space="PSUM") as stps, \
                 tc.tile_pool(name="st_sb", bufs=3) as stsb:
                psum_s = stps.tile([F, b_loc], f32)
                psum_q = stps.tile([F, b_loc], f32)
                n_tiles = (rows_total + 127) // 128
                for i in range(n_tiles):
                    r0 = i * 128
                    nr = min(128, rows_total - r0)
                    ph = r0 % 3
                    xst = stsb.tile([128, b_loc], f16, tag="xst")
                    nc.sync.dma_start(xst[0:nr, :], xT[r0:r0 + nr, :])
                    xsq = stsb.tile([128, b_loc], f16, tag="xsq")
                    nc.vector.tensor_tensor(
                        xsq[0:nr, :], xst[0:nr, :], xst[0:nr, :], Alu.mult)
                    first = i == 0
                    last = i == n_tiles - 1
                    for g in range(NG):
                        cs = slice(g * 512, (g + 1) * 512)
                        nc.tensor.matmul(
                            psum_s[:, cs], eT[0:nr, 3 * ph:3 * ph + 3],
                            xst[0:nr, cs], start=first, stop=last)
                        nc.tensor.matmul(
                            psum_q[:, cs], eT[0:nr, 3 * ph:3 * ph + 3],
                            xsq[0:nr, cs], start=first, stop=last)
                stats_l = stsb.tile([F, 2], f32, tag="stl")
                nc.vector.tensor_reduce(
                    stats_l[:, 0:1], psum_s[:], mybir.AxisListType.X, Alu.add)
                nc.vector.tensor_reduce(
                    stats_l[:, 1:2], psum_q[:], mybir.AxisListType.X, Alu.add)
                # 6-float AllReduce across the 8 cores via DRAM bounce
                cc_in = dpool.tile([F, 2], f32)
                cc_out = dpool.tile([F, 2], f32, addr_space="Shared")
                nc.sync.dma_start(cc_in[:], stats_l[:])
                nc.gpsimd.collective_compute(
                    "AllReduce", Alu.add,
                    replica_groups=[list(range(num_cores))],
                    ins=[cc_in.opt()], outs=[cc_out.opt()])
                nc.sync.dma_start(stats_g[:], cc_out[:])

            # ---- phase 2: BN fold ----------------------------------------
            with tc.tile_pool(name="fold", bufs=1) as fp, \
                 tc.tile_pool(name="fold_ps", bufs=1, space="PSUM") as fps:
                mean = fp.tile([F, 1], f32)
                nc.vector.tensor_scalar(mean[:], stats_g[:, 0:1], inv_bs, None,
                                        Alu.mult)
                msq = fp.tile([F, 1], f32)
                nc.vector.tensor_scalar(msq[:], stats_g[:, 1:2], inv_bs, None,
                                        Alu.mult)
                var = fp.tile([F, 1], f32)
                nc.vector.tensor_tensor(var[:], mean[:], mean[:], Alu.mult)
                nc.vector.tensor_tensor(var[:], msq[:], var[:], Alu.subtract)
                veps = fp.tile([F, 1], f32)
                nc.vector.tensor_scalar(veps[:], var[:], BN_EPS, None, Alu.add)
                # rsqrt: ACT sqrt seed + DVE reciprocal + 2 Newton iters
                sq = fp.tile([F, 1], f32)
                nc.scalar.activation(sq[:], veps[:], Act.Sqrt)
                y = fp.tile([F, 1], f32)
                nc.vector.reciprocal(y[:], sq[:])
                t1 = fp.tile([F, 1], f32)
                t2 = fp.tile([F, 1], f32)
                for _ in range(2):
                    nc.vector.tensor_tensor(t1[:], y[:], y[:], Alu.mult)
                    nc.vector.tensor_tensor(t2[:], t1[:], veps[:], Alu.mult)
                    nc.vector.tensor_scalar(t2[:], t2[:], -0.5, 1.5, Alu.mult,
                                            Alu.add)
                    nc.vector.tensor_tensor(y[:], y[:], t2[:], Alu.mult)
                a_s = fp.tile([F, 1], f32)
                nc.vector.tensor_tensor(a_s[:], y[:], gam[:], Alu.mult)
                b_aff = fp.tile([F, 1], f32)
                nc.vector.tensor_tensor(b_aff[:], mean[:], a_s[:], Alu.mult)
                nc.vector.tensor_tensor(b_aff[:], bet[:], b_aff[:],
                                        Alu.subtract)
                b16 = fp.tile([F, 1], f16)
                nc.vector.tensor_copy(b16[:], b_aff[:])
                pbf = fps.tile([2 * H, 1], f32)
                nc.tensor.matmul(pbf[:], wzx[:], b16[:],
                                 start=True, stop=True)
                nc.vector.tensor_tensor(zb_f[:], bz[:], pbf[:], Alu.add)
                # scaled x-weights at partitions 64..66 (bases must match)
                a67 = fp.tile([H + F, 1], f32)
                nc.sync.dma_start(a67[H:H + F, :], a_s[:])
                nc.sync.dma_start(wzx6[H:H + F, :], wzx_d[:])
                nc.vector.tensor_scalar(wzx6[H:H + F, :], wzx6[H:H + F, :],
                                        a67[H:H + F, 0:1], None, Alu.mult)

            # ---- phase 4: the scan ---------------------------------------
            # State between steps is the UNSUMMED blend pair rz=[u1;u2]
            # ([128,B]); wzdup's duplicated rows compute Wh@(u1+u2). The x and
            # gate inputs live in a second stream tile gx (c,c^2 at rows 0-1;
            # x_t at rows 64-66) whose two matmuls use disjoint row groups.
            # Per step the scalar engine runs one [128,1024] SIGMOID (both
            # chains' gates, 2-bank PSUM tile) + two [128,512] TANHs; the
            # gate/x matmuls for step s+1 are issued before the h matmuls so
            # the serial loop is just ACT -> blend -> K=128 matmul.
            with tc.tile_pool(name="rz", bufs=2) as rzp, \
                 tc.tile_pool(name="gx", bufs=5) as gxp, \
                 tc.tile_pool(name="tg", bufs=2) as tgp, \
                 tc.tile_pool(name="gg", bufs=2) as ggp, \
                 tc.tile_pool(name="ps_za", bufs=1, space="PSUM") as pza, \
                 tc.tile_pool(name="ps_zb", bufs=1, space="PSUM") as pzb, \
                 tc.tile_pool(name="ps_g", bufs=2, space="PSUM") as pgp:
                zpools = [pza, pzb]
                rzs = []
                for c in range(NG):
                    rz = rzp.tile([2 * H, 512], f16, tag=f"rz{c}")
                    nc.vector.memset(rz[:], 0.0)
                    rzs.append(rz)

                def gx_tile(s):
                    gx = gxp.tile([H + F, b_loc], f16, tag="gx")
                    nc.sync.dma_start(
                        gx[0:2, :],
                        cgd[s:s + 1, :].rearrange("a (p n) -> (a p) n", p=2))
                    nc.sync.dma_start(gx[H:H + F, :],
                                      xT[F * s:F * s + F, :])
                    return gx

                def gate_x_mms(s, gx):
                    # interleave gate (q0 rows) and x (q64 rows) so walrus
                    # pairs adjacent disjoint-row-group matmuls on the PE
                    pg = pgp.tile([2 * H, b_loc], f32, tag="pg")
                    pzn = []
                    for c in range(NG):
                        cs = slice(c * 512, (c + 1) * 512)
                        nc.tensor.matmul(pg[:, cs], wg2[:], gx[0:2, cs],
                                         start=True, stop=True)
                        pz = zpools[s % 2].tile([2 * H, 512], f32,
                                                tag=f"pz{c}")
                        nc.tensor.matmul(pz[:], wzx6[H:H + F, :],
                                         gx[H:H + F, cs],
                                         start=True, stop=False)
                        pzn.append(pz)
                    return pg, pzn

                # pipeline prime: steps 0 and 1 gate/x preacts + G(0)
                gx0 = gx_tile(0)
                gx_a = gx_tile(1)          # feeds gates/x of step 1
                gx_b = gx_tile(2)          # feeds gates/x of step 2
                pg_c, pzs = gate_x_mms(0, gx0)
                for c in range(NG):
                    nc.tensor.matmul(pzs[c][:], wzdup[:], rzs[c][:],
                                     start=False, stop=True)
                G_cur = ggp.tile([2 * H, b_loc], f16, tag="G")
                nc.scalar.activation(G_cur[:], pg_c[:], Act.Sigmoid,
                                     bias=cgb2[:])
                pg_n, pz_n = gate_x_mms(1, gx_a)

                for s in range(s_steps):
                    # critical chain first: tanh -> blend -> h-matmul(s+1)
                    for c in range(NG):
                        cs = slice(c * 512, (c + 1) * 512)
                        T = tgp.tile([2 * H, 512], f16, tag=f"T{c}")
                        nc.scalar.activation(T[:], pzs[c][:], Act.Tanh,
                                             bias=zb_f[:])
                        rz_new = rzp.tile([2 * H, 512], f16, tag=f"rz{c}")
                        nc.vector.tensor_tensor(rz_new[:], T[:],
                                                G_cur[:, cs], Alu.mult)
                        rzs[c] = rz_new
                        if s + 1 < s_steps:
                            nc.tensor.matmul(pz_n[c][:], wzdup[:],
                                             rz_new[:],
                                             start=False, stop=True)
                    if s + 1 < s_steps:
                        G_n = ggp.tile([2 * H, b_loc], f16, tag="G")
                        nc.scalar.activation(G_n[:], pg_n[:], Act.Sigmoid,
                                             bias=cgb2[:])
                    # lookahead work (lower scheduler priority than the
                    # chain): gate/x preacts for s+2, stream dma for s+3
                    if s + 2 < s_steps:
                        pg_2, pz_2 = gate_x_mms(s + 2, gx_b)
                    if s + 3 < s_steps:
                        gx_pf = gx_tile(s + 3)
                    if s + 1 < s_steps:
                        pzs = pz_n
                        G_cur = G_n
                        if s + 2 < s_steps:
                            pz_n = pz_2
                            pg_n = pg_2
                            gx_b = gx_pf if s + 3 < s_steps else None

                # ---- phase 5: classifier (per chain) ---------------------
                for c in range(NG):
                    cs = slice(c * 512, (c + 1) * 512)
                    ph = pza.tile([H, 512], f32, tag="pz0")
                    nc.tensor.matmul(ph[:], idup[:], rzs[c][:],
                                     start=True, stop=True)
                    hf = tgp.tile([H, 512], f16, tag=f"T{c}")
                    nc.scalar.copy(hf[:], ph[:])
                    pcls = pza.tile([2, 512], f32, tag="pz1")
                    nc.tensor.matmul(pcls[:], wcls[:], hf[:],
                                     start=True, stop=True)
                    pr = tgp.tile([2, 512], f32, tag=f"T{c}")
                    nc.scalar.activation(pr[:], pcls[:], Act.Sigmoid,
                                         bias=bcls[:])
                    nc.sync.dma_start(probs_d[:, cs], pr[:])

    nc.compile()
    return nc


def _host_prep(inputs, num_cores, s_steps, b_loc):
    """Layout-only host prep: shard batch, transpose to feature-major,
    stack/transpose weights. No data-dependent arithmetic."""
    x = np.asarray(inputs["x"], dtype=np.float32)
    times = np.asarray(inputs["times"], dtype=np.float32)
    ff1_w = np.asarray(inputs["ff1_w"], np.float32)
    ff2_w = np.asarray(inputs["ff2_w"], np.float32)
    ff1_b = np.asarray(inputs["ff1_b"], np.float32)
    ff2_b = np.asarray(inputs["ff2_b"], np.float32)
    ta_w = np.asarray(inputs["ta_w"], np.float32)
    ta_b = np.asarray(inputs["ta_b"], np.float32)
    tb_w = np.asarray(inputs["tb_w"], np.float32)
    tb_b = np.asarray(inputs["tb_b"], np.float32)
    cls_w = np.asarray(inputs["cls_w"], np.float32)
    cls_b = np.asarray(inputs["cls_b"], np.float32)
    gam = np.asarray(inputs["bn_gamma"], np.float32)
    bet = np.asarray(inputs["bn_beta"], np.float32)

    Wst = np.concatenate([ff1_w, ff2_w], 0)  # [128, 67]
    whT = np.ascontiguousarray(Wst[:, F:].T)       # [64, 128]
    wzdup = np.concatenate([whT, whT], 0).astype(np.float16)
    wzx = np.ascontiguousarray(Wst[:, :F].T).astype(np.float16)  # [3, 128]
    idup = np.concatenate([np.eye(H), np.eye(H)], 0).astype(np.float16)
    bz = np.concatenate([ff1_b, ff2_b]).reshape(2 * H, 1).astype(np.float32)
    A = ta_w[:, 0]
    Bc = ta_b + tb_w[:, 0]
    Cc = tb_b
    wg2 = np.stack([np.concatenate([Bc, -Bc]),
                    np.concatenate([A, -A])], 0).astype(np.float16)
    cgb2 = np.concatenate([Cc, -Cc]).reshape(2 * H, 1).astype(np.float32)
    d0 = cls_w[0] - cls_w[1]
    wcls = np.stack([d0, -d0], 1).astype(np.float16)  # [64, 2]
    bcls = np.array([cls_b[0] - cls_b[1],
                     cls_b[1] - cls_b[0]]).reshape(2, 1).astype(np.float32)
    k = np.arange(128)
    eT = np.concatenate(
        [((p + k[:, None]) % 3 == np.arange(3)[None, :]) for p in range(3)],
        axis=1).astype(np.float16)  # [128, 9]

    shared = dict(
        wzdup=wzdup, wzx=wzx, idup=idup, bz=bz, wg2=wg2, cgb2=cgb2,
        eT=eT,
        gam=gam.reshape(F, 1).astype(np.float32),
        bet=bet.reshape(F, 1).astype(np.float32),
        wcls=wcls, bcls=bcls)

    in_maps = []
    for c in range(num_cores):
        sl = slice(c * b_loc, (c + 1) * b_loc)
        xc = x[sl, :s_steps, :]                       # [b_loc, S, 3]
        xT = np.ascontiguousarray(xc.transpose(1, 2, 0)).reshape(
            s_steps * F, b_loc).astype(np.float16)
        tTc = np.ascontiguousarray(times[sl, :s_steps, 0].T).astype(np.float16)
        in_maps.append(dict(shared, xT=xT, tT=tTc))
    return in_maps


def kernel(**inputs):
    import time
    from concourse.bass_utils import run_bass_kernel_spmd

    num_cores, s_steps, b_loc = NUM_CORES, S_FULL, B_FULL // NUM_CORES
    key = (num_cores, s_steps, b_loc)
    if key not in _CACHE:
        _CACHE[key] = _build(*key)
    nc = _CACHE[key]
    in_maps = _host_prep(inputs, num_cores, s_steps, b_loc)
    res = None
    for attempt in range(3):
        try:
            res = run_bass_kernel_spmd(nc, in_maps,
                                       core_ids=list(range(num_cores)))
            break
        except Exception:
            if attempt == 2:
                raise
            time.sleep(5.0)  # transient NRT exec-unit errors recover on retry
    out = np.empty((num_cores * b_loc, 2), np.float32)
    for c in range(num_cores):
        out[c * b_loc:(c + 1) * b_loc] = res.results[c]["probs"].T
    return out


# revision 9
# speedup vs baseline: 1.0786x; 1.0786x over previous
# Trainium2 Bass kernel for nn_LiquidMalwareDetector.
#
# Strategy: pure data parallelism over the batch dim (8192 -> 1024 per core,
# 8 cores). Host side only reshapes/shards (layout, no arithmetic); all math
# runs on device:
#   phase 1: per-core BN channel sums/sumsq (matmul-with-0/1-matrix reduction)
#            + 8-core AllReduce of the 6 partial stats
#   phase 2: fold BN affine into the x-columns of the ff weights + bias
#   phase 3: precompute clipped time c and c^2 for the gate (DRAM staging)
#   phase 4: 1024-step recurrence in feature-major layout, two independent
#            512-column batch chains interleaved to hide the serial latency.
#            State is the unsummed blend pair rz=[g*t1; (1-g)*t2]; duplicated
#            Wh rows in the K=128 matmul sum the pair (walrus forbids SBUF
#            tensor_tensor with mismatched base partitions, so the matmul
#            does the cross-partition add). Gate: K=2 matmul with +/-
#            stacked rows -> sigmoid gives [g; 1-g]; blend = 1 DVE mult.
#   phase 5: classifier via sigmoid of logit-difference (== 2-class softmax)
import numpy as np

NUM_CORES = 8
B_FULL = 8192
S_FULL = 1024
F = 3
H = 64
BN_EPS = 1e-5

_CACHE = {}


def _build(num_cores, s_steps, b_loc):
    from concourse import bacc, mybir
    import concourse.tile as tile

    f16 = mybir.dt.float16
    f32 = mybir.dt.float32
    Alu = mybir.AluOpType
    Act = mybir.ActivationFunctionType

    NG = b_loc // 512  # 512-column groups per step
    assert b_loc % 512 == 0

    nc = bacc.Bacc(
        "TRN2",
        target_bir_lowering=False,
        debug=False,
        num_devices=num_cores,
    )

    # ---- I/O -------------------------------------------------------------
    xT = nc.dram_tensor("xT", [s_steps * F, b_loc], f16, kind="ExternalInput")
    tT = nc.dram_tensor("tT", [s_steps, b_loc], f16, kind="ExternalInput")
    # wzdup: [128,128] duplicated h-weights (rows 0-63 == rows 64-127 == Wh.T)
    # so the matmul itself sums the unsummed blend halves [u1; u2].
    wzdup_d = nc.dram_tensor("wzdup", [2 * H, 2 * H], f16, kind="ExternalInput")
    wzx_d = nc.dram_tensor("wzx", [F, 2 * H], f16, kind="ExternalInput")
    idup_d = nc.dram_tensor("idup", [2 * H, H], f16, kind="ExternalInput")
    bz_d = nc.dram_tensor("bz", [2 * H, 1], f32, kind="ExternalInput")
    wg2_d = nc.dram_tensor("wg2", [2, 2 * H], f16, kind="ExternalInput")
    cgb2_d = nc.dram_tensor("cgb2", [2 * H, 1], f32, kind="ExternalInput")
    eT_d = nc.dram_tensor("eT", [128, 9], f16, kind="ExternalInput")
    gam_d = nc.dram_tensor("gam", [F, 1], f32, kind="ExternalInput")
    bet_d = nc.dram_tensor("bet", [F, 1], f32, kind="ExternalInput")
    wcls_d = nc.dram_tensor("wcls", [H, 2], f16, kind="ExternalInput")
    bcls_d = nc.dram_tensor("bcls", [2, 1], f32, kind="ExternalInput")
    probs_d = nc.dram_tensor("probs", [2, b_loc], f32, kind="ExternalOutput")

    inv_bs = 1.0 / float(num_cores * b_loc * s_steps)

    with tile.TileContext(nc) as tc:
        with tc.tile_pool(name="const", bufs=1) as cpool, \
             tc.tile_pool(name="dram", bufs=1, space="DRAM") as dpool:
            # persistent SBUF-resident weights/constants
            wzdup = cpool.tile([2 * H, 2 * H], f16)
            nc.sync.dma_start(wzdup[:], wzdup_d[:])
            wzx = cpool.tile([F, 2 * H], f16)
            nc.sync.dma_start(wzx[:], wzx_d[:])
            # folded x-weights, placed at partitions 64..66 so the x matmul's
            # lhsT/rhs base partitions match (and auto row-group = 2)
            wzx6 = cpool.tile([H + F + 1, 2 * H], f16)
            idup = cpool.tile([2 * H, H], f16)
            nc.sync.dma_start(idup[:], idup_d[:])
            bz = cpool.tile([2 * H, 1], f32)
            nc.sync.dma_start(bz[:], bz_d[:])
            wg2 = cpool.tile([2, 2 * H], f16)
            nc.sync.dma_start(wg2[:], wg2_d[:])
            cgb2 = cpool.tile([2 * H, 1], f32)
            nc.sync.dma_start(cgb2[:], cgb2_d[:])
            eT = cpool.tile([128, 9], f16)
            nc.sync.dma_start(eT[:], eT_d[:])
            gam = cpool.tile([F, 1], f32)
            nc.sync.dma_start(gam[:], gam_d[:])
            bet = cpool.tile([F, 1], f32)
            nc.sync.dma_start(bet[:], bet_d[:])
            wcls = cpool.tile([H, 2], f16)
            nc.sync.dma_start(wcls[:], wcls_d[:])
            bcls = cpool.tile([2, 1], f32)
            nc.sync.dma_start(bcls[:], bcls_d[:])
            zb_f = cpool.tile([2 * H, 1], f32)   # folded tanh bias
            stats_g = cpool.tile([F, 2], f32)    # all-reduced [sum, sumsq]

            # ---- phase 1 + phase 3 fused ---------------------------------
            # BN stats (sync-queue DMAs, PE reduction) interleaved with the
            # gate input precompute (scalar-queue DMAs, DVE clip/square) so
            # both DMA queues and all engines run concurrently.
            cgd = dpool.tile([s_steps, 2 * b_loc], f16)
            rows_total = s_steps * F
            with tc.tile_pool(name="st_ps", bufs=1, space="PSUM") as stps, \
                 tc.tile_pool(name="st_sb", bufs=3) as stsb, \
                 tc.tile_pool(name="cg", bufs=3) as cgp:
                psum_s = stps.tile([F, b_loc], f32)
                psum_q = stps.tile([F, b_loc], f32)
                n_tiles = (rows_total + 127) // 128
                n_ct = (s_steps + 127) // 128

                def phase3_iter(j):
                    r0 = j * 128
                    nr = min(128, s_steps - r0)
                    ttl = cgp.tile([128, b_loc], f16, tag="tt")
                    nc.scalar.dma_start(ttl[0:nr, :], tT[r0:r0 + nr, :])
                    cc2 = cgp.tile([128, 2 * b_loc], f16, tag="cc2")
                    nc.vector.tensor_scalar(cc2[0:nr, 0:b_loc], ttl[0:nr, :],
                                            60.0, 0.0, Alu.min, Alu.max)
                    nc.vector.tensor_tensor(
                        cc2[0:nr, b_loc:2 * b_loc], cc2[0:nr, 0:b_loc],
                        cc2[0:nr, 0:b_loc], Alu.mult)
                    nc.scalar.dma_start(cgd[r0:r0 + nr, :], cc2[0:nr, :])

                for i in range(n_tiles):
                    r0 = i * 128
                    nr = min(128, rows_total - r0)
                    ph = r0 % 3
                    xst = stsb.tile([128, b_loc], f16, tag="xst")
                    nc.sync.dma_start(xst[0:nr, :], xT[r0:r0 + nr, :])
                    xsq = stsb.tile([128, b_loc], f16, tag="xsq")
                    nc.vector.tensor_tensor(
                        xsq[0:nr, :], xst[0:nr, :], xst[0:nr, :], Alu.mult)
                    first = i == 0
                    last = i == n_tiles - 1
                    for g in range(NG):
                        cs = slice(g * 512, (g + 1) * 512)
                        nc.tensor.matmul(
                            psum_s[:, cs], eT[0:nr, 3 * ph:3 * ph + 3],
                            xst[0:nr, cs], start=first, stop=last)
                        nc.tensor.matmul(
                            psum_q[:, cs], eT[0:nr, 3 * ph:3 * ph + 3],
                            xsq[0:nr, cs], start=first, stop=last)
                    if i % 3 == 2 and i // 3 < n_ct:
                        phase3_iter(i // 3)
                stats_l = stsb.tile([F, 2], f32, tag="stl")
                nc.vector.tensor_reduce(
                    stats_l[:, 0:1], psum_s[:], mybir.AxisListType.X, Alu.add)
                nc.vector.tensor_reduce(
                    stats_l[:, 1:2], psum_q[:], mybir.AxisListType.X, Alu.add)
                # 6-float AllReduce across the 8 cores via DRAM bounce
                cc_in = dpool.tile([F, 2], f32)
                cc_out = dpool.tile([F, 2], f32, addr_space="Shared")
                nc.sync.dma_start(cc_in[:], stats_l[:])
                nc.gpsimd.collective_compute(
                    "AllReduce", Alu.add,
                    replica_groups=[list(range(num_cores))],
                    ins=[cc_in.opt()], outs=[cc_out.opt()])
                nc.sync.dma_start(stats_g[:], cc_out[:])

            # ---- phase 2: BN fold ----------------------------------------
            with tc.tile_pool(name="fold", bufs=1) as fp, \
                 tc.tile_pool(name="fold_ps", bufs=1, space="PSUM") as fps:
                mean = fp.tile([F, 1], f32)
                nc.vector.tensor_scalar(mean[:], stats_g[:, 0:1], inv_bs, None,
                                        Alu.mult)
                msq = fp.tile([F, 1], f32)
                nc.vector.tensor_scalar(msq[:], stats_g[:, 1:2], inv_bs, None,
                                        Alu.mult)
                var = fp.tile([F, 1], f32)
                nc.vector.tensor_tensor(var[:], mean[:], mean[:], Alu.mult)
                nc.vector.tensor_tensor(var[:], msq[:], var[:], Alu.subtract)
                veps = fp.tile([F, 1], f32)
                nc.vector.tensor_scalar(veps[:], var[:], BN_EPS, None, Alu.add)
                # rsqrt: ACT sqrt seed + DVE reciprocal + 2 Newton iters
                sq = fp.tile([F, 1], f32)
                nc.scalar.activation(sq[:], veps[:], Act.Sqrt)
                y = fp.tile([F, 1], f32)
                nc.vector.reciprocal(y[:], sq[:])
                t1 = fp.tile([F, 1], f32)
                t2 = fp.tile([F, 1], f32)
                for _ in range(2):
                    nc.vector.tensor_tensor(t1[:], y[:], y[:], Alu.mult)
                    nc.vector.tensor_tensor(t2[:], t1[:], veps[:], Alu.mult)
                    nc.vector.tensor_scalar(t2[:], t2[:], -0.5, 1.5, Alu.mult,
                                            Alu.add)
                    nc.vector.tensor_tensor(y[:], y[:], t2[:], Alu.mult)
                a_s = fp.tile([F, 1], f32)
                nc.vector.tensor_tensor(a_s[:], y[:], gam[:], Alu.mult)
                b_aff = fp.tile([F, 1], f32)
                nc.vector.tensor_tensor(b_aff[:], mean[:], a_s[:], Alu.mult)
                nc.vector.tensor_tensor(b_aff[:], bet[:], b_aff[:],
                                        Alu.subtract)
                b16 = fp.tile([F, 1], f16)
                nc.vector.tensor_copy(b16[:], b_aff[:])
                pbf = fps.tile([2 * H, 1], f32)
                nc.tensor.matmul(pbf[:], wzx[:], b16[:],
                                 start=True, stop=True)
                nc.vector.tensor_tensor(zb_f[:], bz[:], pbf[:], Alu.add)
                # scaled x-weights at partitions 64..66 (bases must match),
                # with the folded bias transposed into row 67 (the rhs gets a
                # matching ones-row) so the tanh needs no ACT bias.
                a67 = fp.tile([H + F, 1], f32)
                nc.sync.dma_start(a67[H:H + F, :], a_s[:])
                nc.sync.dma_start(wzx6[H:H + F, :], wzx_d[:])
                nc.vector.tensor_scalar(wzx6[H:H + F, :], wzx6[H:H + F, :],
                                        a67[H:H + F, 0:1], None, Alu.mult)

            # ---- phase 4: the scan ---------------------------------------
            # State between steps is the UNSUMMED blend pair rz=[u1;u2]
            # ([128,B]); wzdup's duplicated rows compute Wh@(u1+u2). The x and
            # gate inputs live in a second stream tile gx (c,c^2 at rows 0-1;
            # x_t at rows 64-66) whose two matmuls use disjoint row groups.
            # NG independent 512-col batch chains interleave on the engines,
            # breaking the per-step latency chain (MM->tanh->blend->MM).
            with tc.tile_pool(name="rz", bufs=4) as rzp, \
                 tc.tile_pool(name="gx", bufs=8) as gxp, \
                 tc.tile_pool(name="tg", bufs=4) as tgp, \
                 tc.tile_pool(name="ps_z", bufs=2, space="PSUM") as pzp, \
                 tc.tile_pool(name="ps_g", bufs=2, space="PSUM") as pgp:
                rzs = []
                for c in range(NG):
                    rz = rzp.tile([2 * H, 512], f16, tag=f"rz{c}")
                    nc.vector.memset(rz[:], 0.0)
                    rzs.append(rz)

                def gx_tile(s):
                    gx = gxp.tile([H + F, b_loc], f16, tag="gx")
                    nc.sync.dma_start(
                        gx[0:2, :],
                        cgd[s:s + 1, :].rearrange("a (p n) -> (a p) n", p=2))
                    nc.sync.dma_start(gx[H:H + F, :],
                                      xT[F * s:F * s + F, :])
                    return gx

                gx = gx_tile(0)
                for s in range(s_steps):
                    gx_cur = gx
                    if s + 1 < s_steps:
                        gx = gx_tile(s + 1)
                    for c in range(NG):
                        cs = slice(c * 512, (c + 1) * 512)
                        pz = pzp.tile([2 * H, 512], f32, tag=f"pz{c}")
                        pg = pgp.tile([2 * H, 512], f32, tag=f"pg{c}")
                        nc.tensor.matmul(pz[:], wzdup[:], rzs[c][:],
                                         start=True, stop=False)
                        nc.tensor.matmul(pg[:], wg2[:], gx_cur[0:2, cs],
                                         start=True, stop=True)
                        nc.tensor.matmul(pz[:], wzx6[H:H + F, :],
                                         gx_cur[H:H + F, cs],
                                         start=False, stop=True)
                        G = tgp.tile([2 * H, 512], f16, tag=f"G{c}")
                        nc.scalar.activation(G[:], pg[:], Act.Sigmoid,
                                             bias=cgb2[:])
                        T = tgp.tile([2 * H, 512], f16, tag=f"T{c}")
                        nc.scalar.activation(T[:], pz[:], Act.Tanh,
                                             bias=zb_f[:])
                        rz_n = rzp.tile([2 * H, 512], f16, tag=f"rz{c}")
                        nc.vector.tensor_tensor(rz_n[:], T[:], G[:],
                                                Alu.mult)
                        rzs[c] = rz_n

                # ---- phase 5: classifier (per chain) ---------------------
                for c in range(NG):
                    cs = slice(c * 512, (c + 1) * 512)
                    ph = pzp.tile([H, 512], f32, tag="pz0")
                    nc.tensor.matmul(ph[:], idup[:], rzs[c][:],
                                     start=True, stop=True)
                    hf = tgp.tile([H, 512], f16, tag=f"G{c}")
                    nc.scalar.copy(hf[:], ph[:])
                    pcls = pzp.tile([2, 512], f32, tag="pz1")
                    nc.tensor.matmul(pcls[:], wcls[:], hf[:],
                                     start=True, stop=True)
                    pr = tgp.tile([2, 512], f32, tag=f"T{c}")
                    nc.scalar.activation(pr[:], pcls[:], Act.Sigmoid,
                                         bias=bcls[:])
                    nc.sync.dma_start(probs_d[:, cs], pr[:])

    nc.compile()
    return nc


def _host_prep(inputs, num_cores, s_steps, b_loc):
    """Layout-only host prep: shard batch, transpose to feature-major,
    stack/transpose weights. No data-dependent arithmetic."""
    x = np.asarray(inputs["x"], dtype=np.float32)
    times = np.asarray(inputs["times"], dtype=np.float32)
    ff1_w = np.asarray(inputs["ff1_w"], np.float32)
    ff2_w = np.asarray(inputs["ff2_w"], np.float32)
    ff1_b = np.asarray(inputs["ff1_b"], np.float32)
    ff2_b = np.asarray(inputs["ff2_b"], np.float32)
    ta_w = np.asarray(inputs["ta_w"], np.float32)
    ta_b = np.asarray(inputs["ta_b"], np.float32)
    tb_w = np.asarray(inputs["tb_w"], np.float32)
    tb_b = np.asarray(inputs["tb_b"], np.float32)
    cls_w = np.asarray(inputs["cls_w"], np.float32)
    cls_b = np.asarray(inputs["cls_b"], np.float32)
    gam = np.asarray(inputs["bn_gamma"], np.float32)
    bet = np.asarray(inputs["bn_beta"], np.float32)

    Wst = np.concatenate([ff1_w, ff2_w], 0)  # [128, 67]
    whT = np.ascontiguousarray(Wst[:, F:].T)       # [64, 128]
    wzdup = np.concatenate([whT, whT], 0).astype(np.float16)
    wzx = np.ascontiguousarray(Wst[:, :F].T).astype(np.float16)  # [3, 128]
    idup = np.concatenate([np.eye(H), np.eye(H)], 0).astype(np.float16)
    bz = np.concatenate([ff1_b, ff2_b]).reshape(2 * H, 1).astype(np.float32)
    A = ta_w[:, 0]
    Bc = ta_b + tb_w[:, 0]
    Cc = tb_b
    wg2 = np.stack([np.concatenate([Bc, -Bc]),
                    np.concatenate([A, -A])], 0).astype(np.float16)
    cgb2 = np.concatenate([Cc, -Cc]).reshape(2 * H, 1).astype(np.float32)
    d0 = cls_w[0] - cls_w[1]
    wcls = np.stack([d0, -d0], 1).astype(np.float16)  # [64, 2]
    bcls = np.array([cls_b[0] - cls_b[1],
                     cls_b[1] - cls_b[0]]).reshape(2, 1).astype(np.float32)
    k = np.arange(128)
    eT = np.concatenate(
        [((p + k[:, None]) % 3 == np.arange(3)[None, :]) for p in range(3)],
        axis=1).astype(np.float16)  # [128, 9]

    shared = dict(
        wzdup=wzdup, wzx=wzx, idup=idup, bz=bz, wg2=wg2, cgb2=cgb2,
        eT=eT,
        gam=gam.reshape(F, 1).astype(np.float32),
        bet=bet.reshape(F, 1).astype(np.float32),
        wcls=wcls, bcls=bcls)

    in_maps = []
    for c in range(num_cores):
        sl = slice(c * b_loc, (c + 1) * b_loc)
        xc = x[sl, :s_steps, :]                       # [b_loc, S, 3]
        xT = np.ascontiguousarray(xc.transpose(1, 2, 0)).reshape(
            s_steps * F, b_loc).astype(np.float16)
        tTc = np.ascontiguousarray(times[sl, :s_steps, 0].T).astype(np.float16)
        in_maps.append(dict(shared, xT=xT, tT=tTc))
    return in_maps


def kernel(**inputs):
    import time
    from concourse.bass_utils import run_bass_kernel_spmd

    num_cores, s_steps, b_loc = NUM_CORES, S_FULL, B_FULL // NUM_CORES
    key = (num_cores, s_steps, b_loc)
    if key not in _CACHE:
        _CACHE[key] = _build(*key)
    nc = _CACHE[key]
    in_maps = _host_prep(inputs, num_cores, s_steps, b_loc)
    res = None
    for attempt in range(3):
        try:
            res = run_bass_kernel_spmd(nc, in_maps,
                                       core_ids=list(range(num_cores)))
            break
        except Exception:
            if attempt == 2:
                raise
            time.sleep(5.0)  # transient NRT exec-unit errors recover on retry
    out = np.empty((num_cores * b_loc, 2), np.float32)
    for c in range(num_cores):
        out[c * b_loc:(c + 1) * b_loc] = res.results[c]["probs"].T
    return out



# revision 10
# speedup vs baseline: 1.0941x; 1.0144x over previous
# Trainium2 Bass kernel for nn_LiquidMalwareDetector.
#
# Strategy: pure data parallelism over the batch dim (8192 -> 1024 per core,
# 8 cores). Host side only reshapes/shards (layout, no arithmetic); all math
# runs on device:
#   phase 1: per-core BN channel sums/sumsq (matmul-with-0/1-matrix reduction)
#            + 8-core AllReduce of the 6 partial stats
#   phase 2: fold BN affine into the x-columns of the ff weights + bias
#   phase 3: precompute clipped time c and c^2 for the gate (DRAM staging)
#   phase 4: 1024-step recurrence in feature-major layout, two independent
#            512-column batch chains interleaved to hide the serial latency.
#            State is the unsummed blend pair rz=[g*t1; (1-g)*t2]; duplicated
#            Wh rows in the K=128 matmul sum the pair (walrus forbids SBUF
#            tensor_tensor with mismatched base partitions, so the matmul
#            does the cross-partition add). Gate: K=2 matmul with +/-
#            stacked rows -> sigmoid gives [g; 1-g]; blend = 1 DVE mult.
#   phase 5: classifier via sigmoid of logit-difference (== 2-class softmax)
import numpy as np

NUM_CORES = 8
B_FULL = 8192
S_FULL = 1024
F = 3
H = 64
BN_EPS = 1e-5

_CACHE = {}


def _build(num_cores, s_steps, b_loc):
    from concourse import bacc, mybir
    import concourse.tile as tile

    f16 = mybir.dt.float16
    f32 = mybir.dt.float32
    Alu = mybir.AluOpType
    Act = mybir.ActivationFunctionType

    NG = b_loc // 512  # 512-column groups per step
    assert b_loc % 512 == 0

    nc = bacc.Bacc(
        "TRN2",
        target_bir_lowering=False,
        debug=False,
        num_devices=num_cores,
    )

    # ---- I/O -------------------------------------------------------------
    xT = nc.dram_tensor("xT", [s_steps * F, b_loc], f16, kind="ExternalInput")
    tT = nc.dram_tensor("tT", [s_steps, b_loc], f16, kind="ExternalInput")
    # wzdup: [128,128] duplicated h-weights (rows 0-63 == rows 64-127 == Wh.T)
    # so the matmul itself sums the unsummed blend halves [u1; u2].
    wzdup_d = nc.dram_tensor("wzdup", [2 * H, 2 * H], f16, kind="ExternalInput")
    wzx_d = nc.dram_tensor("wzx", [F, 2 * H], f16, kind="ExternalInput")
    idup_d = nc.dram_tensor("idup", [2 * H, H], f16, kind="ExternalInput")
    bz_d = nc.dram_tensor("bz", [2 * H, 1], f32, kind="ExternalInput")
    wg2_d = nc.dram_tensor("wg2", [2, 2 * H], f16, kind="ExternalInput")
    cgb2_d = nc.dram_tensor("cgb2", [2 * H, 1], f32, kind="ExternalInput")
    eT_d = nc.dram_tensor("eT", [128, 9], f16, kind="ExternalInput")
    gam_d = nc.dram_tensor("gam", [F, 1], f32, kind="ExternalInput")
    bet_d = nc.dram_tensor("bet", [F, 1], f32, kind="ExternalInput")
    wcls_d = nc.dram_tensor("wcls", [H, 2], f16, kind="ExternalInput")
    bcls_d = nc.dram_tensor("bcls", [2, 1], f32, kind="ExternalInput")
    probs_d = nc.dram_tensor("probs", [2, b_loc], f32, kind="ExternalOutput")

    inv_bs = 1.0 / float(num_cores * b_loc * s_steps)

    with tile.TileContext(nc) as tc:
        with tc.tile_pool(name="const", bufs=1) as cpool, \
             tc.tile_pool(name="dram", bufs=1, space="DRAM") as dpool:
            # persistent SBUF-resident weights/constants
            wzdup = cpool.tile([2 * H, 2 * H], f16)
            nc.sync.dma_start(wzdup[:], wzdup_d[:])
            wzx = cpool.tile([F, 2 * H], f16)
            nc.sync.dma_start(wzx[:], wzx_d[:])
            # folded x-weights, placed at partitions 64..66 so the x matmul's
            # lhsT/rhs base partitions match (and auto row-group = 2)
            wzx6 = cpool.tile([H + F + 1, 2 * H], f16)
            idup = cpool.tile([2 * H, H], f16)
            nc.sync.dma_start(idup[:], idup_d[:])
            bz = cpool.tile([2 * H, 1], f32)
            nc.sync.dma_start(bz[:], bz_d[:])
            wg2 = cpool.tile([2, 2 * H], f16)
            nc.sync.dma_start(wg2[:], wg2_d[:])
            cgb2 = cpool.tile([2 * H, 1], f32)
            nc.sync.dma_start(cgb2[:], cgb2_d[:])
            eT = cpool.tile([128, 9], f16)
            nc.sync.dma_start(eT[:], eT_d[:])
            gam = cpool.tile([F, 1], f32)
            nc.sync.dma_start(gam[:], gam_d[:])
            bet = cpool.tile([F, 1], f32)
            nc.sync.dma_start(bet[:], bet_d[:])
            wcls = cpool.tile([H, 2], f16)
            nc.sync.dma_start(wcls[:], wcls_d[:])
            bcls = cpool.tile([2, 1], f32)
            nc.sync.dma_start(bcls[:], bcls_d[:])
            zb_f = cpool.tile([2 * H, 1], f32)   # folded tanh bias
            stats_g = cpool.tile([F, 2], f32)    # all-reduced [sum, sumsq]

            # ---- phase 1 + phase 3 fused ---------------------------------
            # BN stats (sync-queue DMAs, PE reduction) interleaved with the
            # gate input precompute (scalar-queue DMAs, DVE clip/square) so
            # both DMA queues and all engines run concurrently.
            cgd = dpool.tile([s_steps, 2 * b_loc], f16)
            rows_total = s_steps * F
            with tc.tile_pool(name="st_ps", bufs=1, space="PSUM") as stps, \
                 tc.tile_pool(name="st_sb", bufs=3) as stsb, \
                 tc.tile_pool(name="cg", bufs=3) as cgp:
                psum_s = stps.tile([F, b_loc], f32)
                psum_q = stps.tile([F, b_loc], f32)
                n_tiles = (rows_total + 127) // 128
                n_ct = (s_steps + 127) // 128

                def phase3_iter(j):
                    r0 = j * 128
                    nr = min(128, s_steps - r0)
                    ttl = cgp.tile([128, b_loc], f16, tag="tt")
                    nc.scalar.dma_start(ttl[0:nr, :], tT[r0:r0 + nr, :])
                    cc2 = cgp.tile([128, 2 * b_loc], f16, tag="cc2")
                    nc.vector.tensor_scalar(cc2[0:nr, 0:b_loc], ttl[0:nr, :],
                                            60.0, 0.0, Alu.min, Alu.max)
                    nc.vector.tensor_tensor(
                        cc2[0:nr, b_loc:2 * b_loc], cc2[0:nr, 0:b_loc],
                        cc2[0:nr, 0:b_loc], Alu.mult)
                    nc.scalar.dma_start(cgd[r0:r0 + nr, :], cc2[0:nr, :])

                for i in range(n_tiles):
                    r0 = i * 128
                    nr = min(128, rows_total - r0)
                    ph = r0 % 3
                    xst = stsb.tile([128, b_loc], f16, tag="xst")
                    nc.sync.dma_start(xst[0:nr, :], xT[r0:r0 + nr, :])
                    xsq = stsb.tile([128, b_loc], f16, tag="xsq")
                    nc.vector.tensor_tensor(
                        xsq[0:nr, :], xst[0:nr, :], xst[0:nr, :], Alu.mult)
                    first = i == 0
                    last = i == n_tiles - 1
                    for g in range(NG):
                        cs = slice(g * 512, (g + 1) * 512)
                        nc.tensor.matmul(
                            psum_s[:, cs], eT[0:nr, 3 * ph:3 * ph + 3],
                            xst[0:nr, cs], start=first, stop=last)
                        nc.tensor.matmul(
                            psum_q[:, cs], eT[0:nr, 3 * ph:3 * ph + 3],
                            xsq[0:nr, cs], start=first, stop=last)
                    if i % 3 == 2 and i // 3 < n_ct:
                        phase3_iter(i // 3)
                stats_l = stsb.tile([F, 2], f32, tag="stl")
                nc.vector.tensor_reduce(
                    stats_l[:, 0:1], psum_s[:], mybir.AxisListType.X, Alu.add)
                nc.vector.tensor_reduce(
                    stats_l[:, 1:2], psum_q[:], mybir.AxisListType.X, Alu.add)
                # 6-float AllReduce across the 8 cores via DRAM bounce
                cc_in = dpool.tile([F, 2], f32)
                cc_out = dpool.tile([F, 2], f32, addr_space="Shared")
                nc.sync.dma_start(cc_in[:], stats_l[:])
                nc.gpsimd.collective_compute(
                    "AllReduce", Alu.add,
                    replica_groups=[list(range(num_cores))],
                    ins=[cc_in.opt()], outs=[cc_out.opt()])
                nc.sync.dma_start(stats_g[:], cc_out[:])

            # ---- phase 2: BN fold ----------------------------------------
            with tc.tile_pool(name="fold", bufs=1) as fp, \
                 tc.tile_pool(name="fold_ps", bufs=1, space="PSUM") as fps:
                mean = fp.tile([F, 1], f32)
                nc.vector.tensor_scalar(mean[:], stats_g[:, 0:1], inv_bs, None,
                                        Alu.mult)
                msq = fp.tile([F, 1], f32)
                nc.vector.tensor_scalar(msq[:], stats_g[:, 1:2], inv_bs, None,
                                        Alu.mult)
                var = fp.tile([F, 1], f32)
                nc.vector.tensor_tensor(var[:], mean[:], mean[:], Alu.mult)
                nc.vector.tensor_tensor(var[:], msq[:], var[:], Alu.subtract)
                veps = fp.tile([F, 1], f32)
                nc.vector.tensor_scalar(veps[:], var[:], BN_EPS, None, Alu.add)
                # rsqrt: ACT sqrt seed + DVE reciprocal + 2 Newton iters
                sq = fp.tile([F, 1], f32)
                nc.scalar.activation(sq[:], veps[:], Act.Sqrt)
                y = fp.tile([F, 1], f32)
                nc.vector.reciprocal(y[:], sq[:])
                t1 = fp.tile([F, 1], f32)
                t2 = fp.tile([F, 1], f32)
                for _ in range(2):
                    nc.vector.tensor_tensor(t1[:], y[:], y[:], Alu.mult)
                    nc.vector.tensor_tensor(t2[:], t1[:], veps[:], Alu.mult)
                    nc.vector.tensor_scalar(t2[:], t2[:], -0.5, 1.5, Alu.mult,
                                            Alu.add)
                    nc.vector.tensor_tensor(y[:], y[:], t2[:], Alu.mult)
                a_s = fp.tile([F, 1], f32)
                nc.vector.tensor_tensor(a_s[:], y[:], gam[:], Alu.mult)
                b_aff = fp.tile([F, 1], f32)
                nc.vector.tensor_tensor(b_aff[:], mean[:], a_s[:], Alu.mult)
                nc.vector.tensor_tensor(b_aff[:], bet[:], b_aff[:],
                                        Alu.subtract)
                b16 = fp.tile([F, 1], f16)
                nc.vector.tensor_copy(b16[:], b_aff[:])
                pbf = fps.tile([2 * H, 1], f32)
                nc.tensor.matmul(pbf[:], wzx[:], b16[:],
                                 start=True, stop=True)
                nc.vector.tensor_tensor(zb_f[:], bz[:], pbf[:], Alu.add)
                # scaled x-weights at partitions 64..66 (bases must match),
                # with the folded bias transposed into row 67 (the rhs gets a
                # matching ones-row) so the tanh needs no ACT bias.
                a67 = fp.tile([H + F, 1], f32)
                nc.sync.dma_start(a67[H:H + F, :], a_s[:])
                nc.sync.dma_start(wzx6[H:H + F, :], wzx_d[:])
                nc.vector.tensor_scalar(wzx6[H:H + F, :], wzx6[H:H + F, :],
                                        a67[H:H + F, 0:1], None, Alu.mult)

            # ---- phase 4: the scan ---------------------------------------
            # State between steps is the UNSUMMED blend pair rz=[u1;u2]
            # ([128,B]); wzdup's duplicated rows compute Wh@(u1+u2). The x and
            # gate inputs live in a second stream tile gx (c,c^2 at rows 0-1;
            # x_t at rows 64-66) whose two matmuls use disjoint row groups.
            # NG independent 512-col batch chains interleave on the engines,
            # breaking the per-step latency chain (MM->tanh->blend->MM).
            with tc.tile_pool(name="rz", bufs=4) as rzp, \
                 tc.tile_pool(name="gx", bufs=8) as gxp, \
                 tc.tile_pool(name="tg", bufs=4) as tgp, \
                 tc.tile_pool(name="ps_z", bufs=2, space="PSUM") as pzp, \
                 tc.tile_pool(name="ps_g", bufs=2, space="PSUM") as pgp:
                rzs = []
                for c in range(NG):
                    rz = rzp.tile([2 * H, 512], f16, tag=f"rz{c}")
                    nc.vector.memset(rz[:], 0.0)
                    rzs.append(rz)

                def gx_tile(s):
                    gx = gxp.tile([H + F, b_loc], f16, tag="gx")
                    nc.sync.dma_start(
                        gx[0:2, :],
                        cgd[s:s + 1, :].rearrange("a (p n) -> (a p) n", p=2))
                    nc.sync.dma_start(gx[H:H + F, :],
                                      xT[F * s:F * s + F, :])
                    return gx

                gx = gx_tile(0)
                for s in range(s_steps):
                    gx_cur = gx
                    if s + 1 < s_steps:
                        gx = gx_tile(s + 1)
                    for c in range(NG):
                        cs = slice(c * 512, (c + 1) * 512)
                        pz = pzp.tile([2 * H, 512], f32, tag=f"pz{c}")
                        pg = pgp.tile([2 * H, 512], f32, tag=f"pg{c}")
                        # x-part first (start) and h-part last (stop): the
                        # x/gate pair has no dep on this step's blend, so
                        # only the K=128 h-matmul sits on the serial
                        # tanh->blend->matmul->tanh loop.
                        nc.tensor.matmul(pz[:], wzx6[H:H + F, :],
                                         gx_cur[H:H + F, cs],
                                         start=True, stop=False)
                        nc.tensor.matmul(pg[:], wg2[:], gx_cur[0:2, cs],
                                         start=True, stop=True)
                        nc.tensor.matmul(pz[:], wzdup[:], rzs[c][:],
                                         start=False, stop=True)
                        G = tgp.tile([2 * H, 512], f16, tag=f"G{c}")
                        nc.scalar.activation(G[:], pg[:], Act.Sigmoid,
                                             bias=cgb2[:])
                        T = tgp.tile([2 * H, 512], f16, tag=f"T{c}")
                        nc.scalar.activation(T[:], pz[:], Act.Tanh,
                                             bias=zb_f[:])
                        rz_n = rzp.tile([2 * H, 512], f16, tag=f"rz{c}")
                        nc.vector.tensor_tensor(rz_n[:], T[:], G[:],
                                                Alu.mult)
                        rzs[c] = rz_n

                # ---- phase 5: classifier (per chain) ---------------------
                for c in range(NG):
                    cs = slice(c * 512, (c + 1) * 512)
                    ph = pzp.tile([H, 512], f32, tag="pz0")
                    nc.tensor.matmul(ph[:], idup[:], rzs[c][:],
                                     start=True, stop=True)
                    hf = tgp.tile([H, 512], f16, tag=f"G{c}")
                    nc.scalar.copy(hf[:], ph[:])
                    pcls = pzp.tile([2, 512], f32, tag="pz1")
                    nc.tensor.matmul(pcls[:], wcls[:], hf[:],
                                     start=True, stop=True)
                    pr = tgp.tile([2, 512], f32, tag=f"T{c}")
                    nc.scalar.activation(pr[:], pcls[:], Act.Sigmoid,
                                         bias=bcls[:])
                    nc.sync.dma_start(probs_d[:, cs], pr[:])

    nc.compile()
    return nc


def _host_prep(inputs, num_cores, s_steps, b_loc):
    """Layout-only host prep: shard batch, transpose to feature-major,
    stack/transpose weights. No data-dependent arithmetic."""
    x = np.asarray(inputs["x"], dtype=np.float32)
    times = np.asarray(inputs["times"], dtype=np.float32)
    ff1_w = np.asarray(inputs["ff1_w"], np.float32)
    ff2_w = np.asarray(inputs["ff2_w"], np.float32)
    ff1_b = np.asarray(inputs["ff1_b"], np.float32)
    ff2_b = np.asarray(inputs["ff2_b"], np.float32)
    ta_w = np.asarray(inputs["ta_w"], np.float32)
    ta_b = np.asarray(inputs["ta_b"], np.float32)
    tb_w = np.asarray(inputs["tb_w"], np.float32)
    tb_b = np.asarray(inputs["tb_b"], np.float32)
    cls_w = np.asarray(inputs["cls_w"], np.float32)
    cls_b = np.asarray(inputs["cls_b"], np.float32)
    gam = np.asarray(inputs["bn_gamma"], np.float32)
    bet = np.asarray(inputs["bn_beta"], np.float32)

    Wst = np.concatenate([ff1_w, ff2_w], 0)  # [128, 67]
    whT = np.ascontiguousarray(Wst[:, F:].T)       # [64, 128]
    wzdup = np.concatenate([whT, whT], 0).astype(np.float16)
    wzx = np.ascontiguousarray(Wst[:, :F].T).astype(np.float16)  # [3, 128]
    idup = np.concatenate([np.eye(H), np.eye(H)], 0).astype(np.float16)
    bz = np.concatenate([ff1_b, ff2_b]).reshape(2 * H, 1).astype(np.float32)
    A = ta_w[:, 0]
    Bc = ta_b + tb_w[:, 0]
    Cc = tb_b
    wg2 = np.stack([np.concatenate([Bc, -Bc]),
                    np.concatenate([A, -A])], 0).astype(np.float16)
    cgb2 = np.concatenate([Cc, -Cc]).reshape(2 * H, 1).astype(np.float32)
    d0 = cls_w[0] - cls_w[1]
    wcls = np.stack([d0, -d0], 1).astype(np.float16)  # [64, 2]
    bcls = np.array([cls_b[0] - cls_b[1],
                     cls_b[1] - cls_b[0]]).reshape(2, 1).astype(np.float32)
    k = np.arange(128)
    eT = np.concatenate(
        [((p + k[:, None]) % 3 == np.arange(3)[None, :]) for p in range(3)],
        axis=1).astype(np.float16)  # [128, 9]

    shared = dict(
        wzdup=wzdup, wzx=wzx, idup=idup, bz=bz, wg2=wg2, cgb2=cgb2,
        eT=eT,
        gam=gam.reshape(F, 1).astype(np.float32),
        bet=bet.reshape(F, 1).astype(np.float32),
        wcls=wcls, bcls=bcls)

    in_maps = []
    for c in range(num_cores):
        sl = slice(c * b_loc, (c + 1) * b_loc)
        xc = x[sl, :s_steps, :]                       # [b_loc, S, 3]
        xT = np.ascontiguousarray(xc.transpose(1, 2, 0)).reshape(
            s_steps * F, b_loc).astype(np.float16)
        tTc = np.ascontiguousarray(times[sl, :s_steps, 0].T).astype(np.float16)
        in_maps.append(dict(shared, xT=xT, tT=tTc))
    return in_maps


def kernel(**inputs):
    import time
    from concourse.bass_utils import run_bass_kernel_spmd

    num_cores, s_steps, b_loc = NUM_CORES, S_FULL, B_FULL // NUM_CORES
    key = (num_cores, s_steps, b_loc)
    if key not in _CACHE:
        _CACHE[key] = _build(*key)
    nc = _CACHE[key]
    in_maps = _host_prep(inputs, num_cores, s_steps, b_loc)
    res = None
    for attempt in range(3):
        try:
            res = run_bass_kernel_spmd(nc, in_maps,
                                       core_ids=list(range(num_cores)))
            break
        except Exception:
            if attempt == 2:
                raise
            time.sleep(5.0)  # transient NRT exec-unit errors recover on retry
    out = np.empty((num_cores * b_loc, 2), np.float32)
    for c in range(num_cores):
        out[c * b_loc:(c + 1) * b_loc] = res.results[c]["probs"].T
    return out

